# revision 12
# baseline (speedup 1.0000x reference)
"""GCN encoder (2x GCNConv + BN/ReLU + fused head) on 8 Trainium2 NeuronCores.

Strategy (edge-parallel, dst-owner): each core owns a contiguous range of
output nodes and processes exactly the edges whose destination falls in its
range.  Edges are sorted by destination tile; the per-tile scatter-add is
expressed as a sequence of one-hot matmuls (S_T built on-device with
is_equal against an iota row) accumulated in PSUM.  Source rows are fetched
with the SWDGE dma_gather instruction (int16 indices relative to one of four
25000-row source groups).  Five SPMD launches with host-side concat (layout
only, no host float math on tensor data):

  L0: degree -> dinv per owned node; x' = x * dinv (bf16); dist/deg scalar stats
  L1: conv1: gather x'[src], S-matmul, @W1, *dinv[dst]  -> h1 (f32) + BN1 sums
  L2: BN1 finalize/apply + ReLU + dinv prescale -> h1nd (bf16)
  L3: conv2 (same program as L1): gather h1nd, @W2 -> h2 (f32) + BN2 sums
  L4: BN2 apply + ReLU + head (h2n@Wm_h + rank-1 dist/deg branches + bm)
"""

import time

import numpy as np
import ml_dtypes

import jax
import jax.numpy as jnp
from jax.sharding import Mesh, PartitionSpec
from jax.experimental.shard_map import shard_map

from concourse import bacc, mybir
import concourse.bass as bass
import concourse.tile as tile
from concourse import bass2jax
from concourse.library_config import mlp

F32 = mybir.dt.float32
BF16 = mybir.dt.bfloat16
I16 = mybir.dt.int16
ALU = mybir.AluOpType
ACTF = mybir.ActivationFunctionType

N = 100000
E = 1600000
F = 128
NCORES = 8
RPC = 12544          # rows per core (98 tiles of 128); core 7 real rows: 12192
NT = 98              # dst tiles per core
TILE = 128
GROUPS = 4           # int16 src index groups
NV = NCORES * RPC   # padded node-id space (relabeled)
GSZ = NV // GROUPS   # 25088 < 32768 (int16 ok)
TB = 8               # dst tiles per gather block
EPS = 1e-5
MAX_GCHUNK = 60      # max chunks per dma_gather instruction (<= 8192 idx limit)

BLOCKS = [list(range(b, min(b + TB, NT))) for b in range(0, NT, TB)]

_bf = ml_dtypes.bfloat16

# ----------------------------------------------------------------------------
# host-side index prep (layout / sorting / padding only -- no tensor math)
# ----------------------------------------------------------------------------


def _relabel(edge_index):
    """degree-balanced node permutation: heavy nodes spread round-robin over
    the 8*98 global tiles (snake order).  Returns new_id[old] in [0, NV)."""
    indeg = np.bincount(edge_index[1].astype(np.int64), minlength=N)
    order = np.argsort(-indeg, kind="stable")
    NTG = NCORES * NT
    pos = np.arange(N)
    rnd = pos // NTG
    tir = pos % NTG
    tilei = np.where(rnd % 2 == 0, tir, NTG - 1 - tir)
    new_global = (tilei // NT) * RPC + (tilei % NT) * TILE + rnd
    new_id = np.empty(N, np.int64)
    new_id[order] = new_global
    return new_id


def _prep_edges(edge_index, edge_weight, new_id):
    src = new_id[edge_index[0].astype(np.int64)]
    dst = new_id[edge_index[1].astype(np.int64)]
    loops = new_id.copy()
    src2 = np.concatenate([src, loops])
    dst2 = np.concatenate([dst, loops])
    ew2 = np.concatenate(
        [edge_weight.astype(np.float32), np.ones(N, np.float32)]
    )
    M = src2.shape[0]

    core = dst2 // RPC
    rloc = dst2 - core * RPC
    t = rloc // TILE
    dloc = (rloc % TILE).astype(np.float32)
    g = src2 // GSZ
    src_rel = (src2 - g * GSZ).astype(np.int16)

    seg = (core * NT + t) * GROUPS + g           # global segment id
    order = np.argsort(seg, kind="stable")
    seg_s = seg[order]
    counts = np.bincount(seg, minlength=NCORES * NT * GROUPS)
    counts_ctg = counts.reshape(NCORES, NT, GROUPS)

    # compile-time chunk map: shared by all cores
    K_tg = -(-counts_ctg.max(axis=0) // TILE)     # [NT, GROUPS] ceil
    for ti in range(NT):
        if K_tg[ti].sum() == 0:
            K_tg[ti][0] = 1
    TOTC = int(K_tg.sum())

    Kflat = K_tg.reshape(-1)                      # [NT*GROUPS] in (t, g) order
    chunk_off = np.concatenate([[0], np.cumsum(Kflat)])[:-1].reshape(NT, GROUPS)

    # slot of each edge: core*TOTC*128 + chunk_off[t,g]*128 + rank_in_segment
    starts = np.concatenate([[0], np.cumsum(counts)])[:-1]
    rank = np.arange(M) - starts[seg_s]
    tg_s = seg_s % (NT * GROUPS)
    slot = seg_s // (NT * GROUPS) * (TOTC * TILE) + chunk_off.reshape(-1)[tg_s] * TILE + rank

    src_slots = np.zeros(NCORES * TOTC * TILE, np.int16)
    ew_slots = np.zeros(NCORES * TOTC * TILE, np.float32)
    dloc_slots = np.zeros(NCORES * TOTC * TILE, np.float32)
    src_slots[slot] = src_rel[order]
    ew_slots[slot] = ew2[order]
    dloc_slots[slot] = dloc[order]
    src_slots = src_slots.reshape(NCORES, TOTC, TILE)
    ew_slots = ew_slots.reshape(NCORES, TOTC, TILE)
    dloc_slots = dloc_slots.reshape(NCORES, TOTC, TILE)

    # meta columns in (t, g, k) order: [cores, 128, TOTC]
    ew_cols = np.ascontiguousarray(np.swapaxes(ew_slots, 1, 2))
    dloc_cols = np.ascontiguousarray(np.swapaxes(dloc_slots, 1, 2))

    # gather chunk lists per (block, g): chunks of tiles in block, split to
    # pieces of <= MAX_GCHUNK chunks
    gather_plan = []      # list over blocks of list over g of list of pieces
    idx_parts = []        # int16 wrapped index arrays, per piece, per core
    for blk in BLOCKS:
        per_g = []
        for gi in range(GROUPS):
            chunk_ids = []
            for ti in blk:
                chunk_ids.extend(
                    range(chunk_off[ti, gi], chunk_off[ti, gi] + K_tg[ti, gi])
                )
            pieces = [
                chunk_ids[i : i + MAX_GCHUNK]
                for i in range(0, len(chunk_ids), MAX_GCHUNK)
            ]
            per_g.append(pieces)
            for piece in pieces:
                flat = src_slots[:, piece, :].reshape(NCORES, -1)  # [8, n*128]
                wrapped = np.tile(
                    flat.reshape(NCORES, -1, 16).swapaxes(1, 2), (1, 8, 1)
                )  # [8, 128, n*8]
                idx_parts.append(wrapped)
        gather_plan.append(per_g)
    idx_all = np.concatenate(idx_parts, axis=2)   # [8, 128, TOTC*8]

    # per-node edge-weight table for degree computation
    full_counts = np.bincount(dst2, minlength=NV)
    ctile = full_counts.reshape(NCORES, NT, TILE)
    KD_t = ctile.max(axis=(0, 2))
    KD_t = np.maximum(KD_t, 1)
    KDoff = np.concatenate([[0], np.cumsum(KD_t)])[:-1]
    KDTOT = int(KD_t.sum())

    o2 = np.argsort(dst2, kind="stable")
    dst_s2 = dst2[o2]
    st2 = np.concatenate([[0], np.cumsum(full_counts)])[:-1]
    rank2 = np.arange(M) - st2[dst_s2]
    c2 = dst_s2 // RPC
    rl2 = dst_s2 - c2 * RPC
    t2 = rl2 // TILE
    p2 = rl2 % TILE
    ewn = np.zeros((NCORES, TILE, KDTOT), np.float32)
    ewn[c2, p2, KDoff[t2] + rank2] = ew2[o2]

    meta = {
        "K_tg": K_tg,
        "chunk_off": chunk_off,
        "TOTC": TOTC,
        "gather_plan": gather_plan,
        "KD_t": KD_t,
        "KDoff": KDoff,
        "KDTOT": KDTOT,
    }
    arrays = {
        "ew_cols": ew_cols,
        "dloc_cols": dloc_cols,
        "idx_all": idx_all,
        "ewn": ewn,
    }
    return meta, arrays


def _scatter_rows(a, new_id):
    """[N, ...] -> [8, RPC, ...]: row old-i lands at new_id[i]."""
    out = np.zeros((NV,) + a.shape[1:], a.dtype)
    out[new_id] = a
    return out.reshape((NCORES, RPC) + a.shape[1:])


def _col_layout(a, new_id):
    """[N] -> [8, 128, NT]  with relabeled node t*128+p at [c, p, t]."""
    padded = np.zeros(NV, np.float32)
    padded[new_id] = a.astype(np.float32)
    return np.ascontiguousarray(
        padded.reshape(NCORES, NT, TILE).swapaxes(1, 2)
    )


# ----------------------------------------------------------------------------
# bass program builders
# ----------------------------------------------------------------------------


def _new_nc():
    return bacc.Bacc("TRN2", target_bir_lowering=False, debug=False,
                     num_devices=NCORES)


def _build_L0(meta):
    KD_t, KDoff, KDTOT = meta["KD_t"], meta["KDoff"], meta["KDTOT"]
    nc = _new_nc()
    x_sh = nc.dram_tensor("x_sh", [RPC, F], F32, kind="ExternalInput")
    ewn = nc.dram_tensor("ewn", [TILE, KDTOT], F32, kind="ExternalInput")
    dist_sh = nc.dram_tensor("dist_sh", [TILE, NT], F32, kind="ExternalInput")
    degf_sh = nc.dram_tensor("degf_sh", [TILE, NT], F32, kind="ExternalInput")
    ones_col = nc.dram_tensor("ones_col", [TILE, 1], F32, kind="ExternalInput")
    dinv_out = nc.dram_tensor("dinv_out", [TILE, NT], F32, kind="ExternalOutput")
    xp_out = nc.dram_tensor("xp_out", [RPC, F], BF16, kind="ExternalOutput")
    st4_out = nc.dram_tensor("st4_out", [1, 4], F32, kind="ExternalOutput")

    with tile.TileContext(nc) as tc:
        with tc.tile_pool(name="sb", bufs=1) as cp, \
             tc.tile_pool(name="work", bufs=3) as wp, \
             tc.tile_pool(name="ps", bufs=2, space="PSUM") as pp:
            ewt = cp.tile([TILE, KDTOT], F32)
            nc.sync.dma_start(out=ewt[:], in_=ewn.ap())
            ones = cp.tile([TILE, 1], F32)
            nc.sync.dma_start(out=ones[:], in_=ones_col.ap())
            dist_t = cp.tile([TILE, NT], F32)
            nc.sync.dma_start(out=dist_t[:], in_=dist_sh.ap())
            degf_t = cp.tile([TILE, NT], F32)
            nc.sync.dma_start(out=degf_t[:], in_=degf_sh.ap())
            dinv_sb = cp.tile([TILE, NT], F32)
            deg_sb = cp.tile([TILE, NT], F32)

            for t in range(NT):
                nc.vector.tensor_reduce(
                    out=deg_sb[:, t : t + 1],
                    in_=ewt[:, int(KDoff[t]) : int(KDoff[t] + KD_t[t])],
                    axis=mybir.AxisListType.X, op=ALU.add)
            m0 = cp.tile([TILE, NT], F32)
            nc.vector.tensor_scalar(out=m0[:], in0=deg_sb[:], scalar1=0.0,
                                    scalar2=None, op0=ALU.is_equal)
            nc.vector.tensor_tensor(out=deg_sb[:], in0=deg_sb[:], in1=m0[:],
                                    op=ALU.add)
            sqd = cp.tile([TILE, NT], F32)
            nc.scalar.activation(sqd[:], deg_sb[:], ACTF.Sqrt)
            nc.vector.reciprocal(out=dinv_sb[:], in_=sqd[:])

            XB = 7  # row-tiles per DMA (98 = 14*7)
            for tb in range(0, NT, XB):
                xt = wp.tile([TILE, XB * F], F32, tag="xt")
                nc.sync.dma_start(
                    out=xt[:].rearrange("p (j f) -> p j f", f=F),
                    in_=x_sh.ap()[tb * TILE : (tb + XB) * TILE, :].rearrange(
                        "(j p) f -> p j f", p=TILE))
                xp = wp.tile([TILE, XB * F], BF16, tag="xp")
                for j in range(XB):
                    nc.scalar.activation(
                        xp[:, j * F : (j + 1) * F], xt[:, j * F : (j + 1) * F],
                        ACTF.Copy, scale=dinv_sb[:, tb + j : tb + j + 1])
                nc.sync.dma_start(
                    out=xp_out.ap()[tb * TILE : (tb + XB) * TILE, :].rearrange(
                        "(j p) f -> p j f", p=TILE),
                    in_=xp[:].rearrange("p (j f) -> p j f", f=F))

            nc.sync.dma_start(out=dinv_out.ap(), in_=dinv_sb[:])

            # scalar-feature stats: columns (sum_d, sumsq_d, sum_g, sumsq_g)
            scols = cp.tile([TILE, 4], F32)
            nc.vector.tensor_reduce(out=scols[:, 0:1], in_=dist_t[:],
                                    axis=mybir.AxisListType.X, op=ALU.add)
            d2 = cp.tile([TILE, NT], F32)
            nc.scalar.activation(d2[:], dist_t[:], ACTF.Square)
            nc.vector.tensor_reduce(out=scols[:, 1:2], in_=d2[:],
                                    axis=mybir.AxisListType.X, op=ALU.add)
            nc.vector.tensor_reduce(out=scols[:, 2:3], in_=degf_t[:],
                                    axis=mybir.AxisListType.X, op=ALU.add)
            g2 = cp.tile([TILE, NT], F32)
            nc.scalar.activation(g2[:], degf_t[:], ACTF.Square)
            nc.vector.tensor_reduce(out=scols[:, 3:4], in_=g2[:],
                                    axis=mybir.AxisListType.X, op=ALU.add)
            sps = pp.tile([1, 4], F32, space="PSUM")
            nc.tensor.matmul(out=sps[:], lhsT=ones[:], rhs=scols[:],
                             start=True, stop=True)
            srow = cp.tile([1, 4], F32)
            nc.vector.tensor_copy(out=srow[:], in_=sps[:])
            nc.sync.dma_start(out=st4_out.ap(), in_=srow[:])
    nc.compile()
    return nc


def _build_conv(meta):
    """Shared program for conv1 (tbl=x', W=W1) and conv2 (tbl=h1nd, W=W2)."""
    K_tg, chunk_off, TOTC = meta["K_tg"], meta["chunk_off"], meta["TOTC"]
    gather_plan = meta["gather_plan"]

    nc = _new_nc()
    tbl = nc.dram_tensor("tbl", [NV, F], BF16, kind="ExternalInput")
    idx_all = nc.dram_tensor("idx_all", [TILE, TOTC * 8], I16, kind="ExternalInput")
    ew_cols = nc.dram_tensor("ew_cols", [TILE, TOTC], F32, kind="ExternalInput")
    dl_cols = nc.dram_tensor("dl_cols", [TILE, TOTC], F32, kind="ExternalInput")
    dinv = nc.dram_tensor("dinv", [TILE, NT], F32, kind="ExternalInput")
    w_in = nc.dram_tensor("w_in", [F, F], F32, kind="ExternalInput")
    iota_in = nc.dram_tensor("iota_in", [TILE, TILE], BF16, kind="ExternalInput")
    h_out = nc.dram_tensor("h_out", [RPC, F], F32, kind="ExternalOutput")
    sum_out = nc.dram_tensor("sum_out", [TILE, F], F32, kind="ExternalOutput")
    sq_out = nc.dram_tensor("sq_out", [TILE, F], F32, kind="ExternalOutput")

    with tile.TileContext(nc) as tc:
        nc.gpsimd.load_library(mlp)
        with tc.tile_pool(name="const", bufs=1) as cp, \
             tc.tile_pool(name="gat", bufs=2) as gp, \
             tc.tile_pool(name="meta", bufs=2) as mp, \
             tc.tile_pool(name="work", bufs=6) as wp, \
             tc.tile_pool(name="acc", bufs=4, space="PSUM") as ap, \
             tc.tile_pool(name="hp", bufs=4, space="PSUM") as hp:
            iota_t = cp.tile([TILE, TILE], BF16)
            nc.sync.dma_start(out=iota_t[:], in_=iota_in.ap())
            w32 = cp.tile([F, F], F32)
            nc.sync.dma_start(out=w32[:], in_=w_in.ap())
            wbf = cp.tile([F, F], BF16)
            nc.vector.tensor_copy(out=wbf[:], in_=w32[:])
            dinv_t = cp.tile([TILE, NT], F32)
            nc.sync.dma_start(out=dinv_t[:], in_=dinv.ap())
            sum_acc = cp.tile([TILE, F], F32)
            nc.vector.memset(sum_acc[:], 0.0)
            sq_acc = cp.tile([TILE, F], F32)
            nc.vector.memset(sq_acc[:], 0.0)

            goff = 0  # running chunk offset inside idx_all
            for bi, blk in enumerate(BLOCKS):
                # gathers for this block, one tile buffer per group
                gts = []
                gpos0 = []  # start chunk (within group buffer) per tile
                for gi in range(GROUPS):
                    pieces = gather_plan[bi][gi]
                    nch = sum(len(p) for p in pieces)
                    if nch == 0:
                        gts.append(None)
                        gpos0.append(None)
                        continue
                    gt = gp.tile([TILE, nch * TILE], BF16, tag=f"g{gi}")
                    pos = 0
                    for piece in pieces:
                        npc = len(piece)
                        it = mp.tile([TILE, npc * 8], I16, tag=f"i{gi}")
                        nc.sync.dma_start(
                            out=it[:],
                            in_=idx_all.ap()[:, goff * 8 : (goff + npc) * 8])
                        base = gi * GSZ
                        top = base + GSZ
                        out_ap = gt[:, pos * F : (pos + npc) * F].rearrange(
                            "p (c d) -> p c d", d=F)
                        nc.gpsimd.dma_gather(
                            out_ap, tbl.ap()[base:top, :], it[:],
                            npc * TILE, npc * TILE, F,
                            single_packet=False,
                        )
                        pos += npc
                        goff += npc
                    gts.append(gt)
                    starts = {}
                    s = 0
                    for ti in blk:
                        starts[ti] = s
                        s += int(K_tg[ti, gi])
                    gpos0.append(starts)

                c0 = int(chunk_off[blk[0], 0])
                cb = int(K_tg[blk, :].sum())
                ewt = mp.tile([TILE, cb], F32, tag="ew")
                nc.sync.dma_start(out=ewt[:], in_=ew_cols.ap()[:, c0 : c0 + cb])
                dlt = mp.tile([TILE, cb], F32, tag="dl")
                nc.sync.dma_start(out=dlt[:], in_=dl_cols.ap()[:, c0 : c0 + cb])

                for ti in blk:
                    ntc = int(K_tg[ti].sum())
                    acc = ap.tile([TILE, TILE], F32, space="PSUM", tag="acc")
                    j = 0
                    for gi in range(GROUPS):
                        kk = int(K_tg[ti, gi])
                        for k in range(kk):
                            col = int(chunk_off[ti, gi]) + k - c0
                            st = wp.tile([TILE, TILE], BF16, tag="st")
                            nc.vector.tensor_scalar(
                                out=st[:], in0=iota_t[:],
                                scalar1=dlt[:, col : col + 1],
                                scalar2=ewt[:, col : col + 1],
                                op0=ALU.is_equal, op1=ALU.mult)
                            gslice = gts[gi][:, (gpos0[gi][ti] + k) * F
                                             : (gpos0[gi][ti] + k + 1) * F]
                            nc.tensor.matmul(out=acc[:], lhsT=gslice, rhs=st[:],
                                             start=(j == 0), stop=(j == ntc - 1))
                            j += 1
                    accs = wp.tile([TILE, TILE], BF16, tag="accs")
                    nc.vector.tensor_copy(out=accs[:], in_=acc[:])
                    h_ps = hp.tile([TILE, F], F32, space="PSUM", tag="h")
                    nc.tensor.matmul(out=h_ps[:], lhsT=accs[:], rhs=wbf[:],
                                     start=True, stop=True)
                    h32 = wp.tile([TILE, F], F32, tag="h32")
                    nc.vector.tensor_scalar(
                        out=h32[:], in0=h_ps[:],
                        scalar1=dinv_t[:, ti : ti + 1], scalar2=None,
                        op0=ALU.mult)
                    nc.vector.tensor_tensor(out=sum_acc[:], in0=sum_acc[:],
                                            in1=h32[:], op=ALU.add)
                    hsq = wp.tile([TILE, F], F32, tag="hsq")
                    nc.scalar.activation(hsq[:], h32[:], ACTF.Square)
                    nc.vector.tensor_tensor(out=sq_acc[:], in0=sq_acc[:],
                                            in1=hsq[:], op=ALU.add)
                    nc.sync.dma_start(
                        out=h_out.ap()[ti * TILE : (ti + 1) * TILE, :],
                        in_=h32[:])

            nc.sync.dma_start(out=sum_out.ap(), in_=sum_acc[:])
            nc.sync.dma_start(out=sq_out.ap(), in_=sq_acc[:])
    nc.compile()
    return nc


def _bn_finalize(nc, cp, pp, sums_t, sqs_t, g_row, b_row, ones, ones_row):
    """device-side BN scale/offset from stacked per-core partial sums.

    Returns (s_b, o_b): [128,128] broadcast tiles (f32, SBUF).
    sums_t/sqs_t: input DRAM tensors [8*128, 128].
    """
    tot_s = cp.tile([TILE, F], F32, tag="bn_ts")
    tot_q = cp.tile([TILE, F], F32, tag="bn_tq")
    for i in range(NCORES):
        a = cp.tile([TILE, F], F32, tag="bn_a")
        nc.sync.dma_start(out=a[:], in_=sums_t.ap()[i * TILE : (i + 1) * TILE, :])
        if i == 0:
            nc.vector.tensor_copy(out=tot_s[:], in_=a[:])
        else:
            nc.vector.tensor_tensor(out=tot_s[:], in0=tot_s[:], in1=a[:], op=ALU.add)
        b = cp.tile([TILE, F], F32, tag="bn_b")
        nc.sync.dma_start(out=b[:], in_=sqs_t.ap()[i * TILE : (i + 1) * TILE, :])
        if i == 0:
            nc.vector.tensor_copy(out=tot_q[:], in_=b[:])
        else:
            nc.vector.tensor_tensor(out=tot_q[:], in0=tot_q[:], in1=b[:], op=ALU.add)
    cs = pp.tile([1, F], F32, space="PSUM", tag="pro")
    nc.tensor.matmul(out=cs[:], lhsT=ones[:], rhs=tot_s[:], start=True, stop=True)
    mu = cp.tile([1, F], F32, tag="bn_mu")
    nc.vector.tensor_scalar(out=mu[:], in0=cs[:], scalar1=1.0 / N, scalar2=None,
                            op0=ALU.mult)
    cq = pp.tile([1, F], F32, space="PSUM", tag="pro")
    nc.tensor.matmul(out=cq[:], lhsT=ones[:], rhs=tot_q[:], start=True, stop=True)
    msq = cp.tile([1, F], F32, tag="bn_msq")
    nc.vector.tensor_scalar(out=msq[:], in0=cq[:], scalar1=1.0 / N, scalar2=None,
                            op0=ALU.mult)
    var = cp.tile([1, F], F32, tag="bn_var")
    nc.vector.tensor_tensor(out=var[:], in0=mu[:], in1=mu[:], op=ALU.mult)
    nc.vector.tensor_tensor(out=var[:], in0=msq[:], in1=var[:], op=ALU.subtract)
    nc.vector.tensor_scalar(out=var[:], in0=var[:], scalar1=EPS, scalar2=None,
                            op0=ALU.add)
    sv = cp.tile([1, F], F32, tag="bn_sv")
    nc.scalar.activation(sv[:], var[:], ACTF.Sqrt)
    rs = cp.tile([1, F], F32, tag="bn_rs")
    nc.vector.reciprocal(out=rs[:], in_=sv[:])
    s1 = cp.tile([1, F], F32, tag="bn_s1")
    nc.vector.tensor_tensor(out=s1[:], in0=g_row[:], in1=rs[:], op=ALU.mult)
    o1 = cp.tile([1, F], F32, tag="bn_o1")
    nc.vector.tensor_tensor(out=o1[:], in0=mu[:], in1=s1[:], op=ALU.mult)
    nc.vector.tensor_tensor(out=o1[:], in0=b_row[:], in1=o1[:], op=ALU.subtract)
    sb_ps = pp.tile([TILE, F], F32, space="PSUM", tag="pro")
    nc.tensor.matmul(out=sb_ps[:], lhsT=ones_row[:], rhs=s1[:], start=True, stop=True)
    s_b = cp.tile([TILE, F], F32, tag="bn_sb")
    nc.vector.tensor_copy(out=s_b[:], in_=sb_ps[:])
    ob_ps = pp.tile([TILE, F], F32, space="PSUM", tag="pro")
    nc.tensor.matmul(out=ob_ps[:], lhsT=ones_row[:], rhs=o1[:], start=True, stop=True)
    o_b = cp.tile([TILE, F], F32, tag="bn_ob")
    nc.vector.tensor_copy(out=o_b[:], in_=ob_ps[:])
    return s_b, o_b


def _build_L2(meta):
    nc = _new_nc()
    h1_sh = nc.dram_tensor("h1_sh", [RPC, F], F32, kind="ExternalInput")
    sums = nc.dram_tensor("sums", [NCORES * TILE, F], F32, kind="ExternalInput")
    sqs = nc.dram_tensor("sqs", [NCORES * TILE, F], F32, kind="ExternalInput")
    bn_g = nc.dram_tensor("bn_g", [1, F], F32, kind="ExternalInput")
    bn_b = nc.dram_tensor("bn_b", [1, F], F32, kind="ExternalInput")
    dinv = nc.dram_tensor("dinv", [TILE, NT], F32, kind="ExternalInput")
    ones_col = nc.dram_tensor("ones_col", [TILE, 1], F32, kind="ExternalInput")
    ones_row = nc.dram_tensor("ones_row", [1, TILE], F32, kind="ExternalInput")
    hn_out = nc.dram_tensor("hn_out", [RPC, F], BF16, kind="ExternalOutput")

    with tile.TileContext(nc) as tc:
        with tc.tile_pool(name="c", bufs=1) as cp, \
             tc.tile_pool(name="w", bufs=3) as wp, \
             tc.tile_pool(name="ps", bufs=2, space="PSUM") as pp:
            ones = cp.tile([TILE, 1], F32)
            nc.sync.dma_start(out=ones[:], in_=ones_col.ap())
            onesr = cp.tile([1, TILE], F32)
            nc.sync.dma_start(out=onesr[:], in_=ones_row.ap())
            g_row = cp.tile([1, F], F32)
            nc.sync.dma_start(out=g_row[:], in_=bn_g.ap())
            b_row = cp.tile([1, F], F32)
            nc.sync.dma_start(out=b_row[:], in_=bn_b.ap())
            dinv_t = cp.tile([TILE, NT], F32)
            nc.sync.dma_start(out=dinv_t[:], in_=dinv.ap())

            s_b, o_b = _bn_finalize(nc, cp, pp, sums, sqs, g_row, b_row,
                                    ones, onesr)

            XB = 7
            for tb in range(0, NT, XB):
                ht = wp.tile([TILE, XB * F], F32, tag="ht")
                nc.sync.dma_start(
                    out=ht[:].rearrange("p (j f) -> p j f", f=F),
                    in_=h1_sh.ap()[tb * TILE : (tb + XB) * TILE, :].rearrange(
                        "(j p) f -> p j f", p=TILE))
                hn = wp.tile([TILE, XB * F], BF16, tag="hn")
                for j in range(XB):
                    t1 = wp.tile([TILE, F], F32, tag="t1")
                    nc.vector.tensor_tensor(
                        out=t1[:], in0=ht[:, j * F : (j + 1) * F], in1=s_b[:],
                        op=ALU.mult)
                    nc.vector.tensor_tensor(out=t1[:], in0=t1[:], in1=o_b[:],
                                            op=ALU.add)
                    nc.scalar.activation(
                        hn[:, j * F : (j + 1) * F], t1[:], ACTF.Relu,
                        scale=dinv_t[:, tb + j : tb + j + 1])
                nc.sync.dma_start(
                    out=hn_out.ap()[tb * TILE : (tb + XB) * TILE, :].rearrange(
                        "(j p) f -> p j f", p=TILE),
                    in_=hn[:].rearrange("p (j f) -> p j f", f=F))
    nc.compile()
    return nc


def _build_L4(meta):
    nc = _new_nc()
    h2_sh = nc.dram_tensor("h2_sh", [RPC, F], F32, kind="ExternalInput")
    sums = nc.dram_tensor("sums", [NCORES * TILE, F], F32, kind="ExternalInput")
    sqs = nc.dram_tensor("sqs", [NCORES * TILE, F], F32, kind="ExternalInput")
    bn_g = nc.dram_tensor("bn_g", [1, F], F32, kind="ExternalInput")
    bn_b = nc.dram_tensor("bn_b", [1, F], F32, kind="ExternalInput")
    st4 = nc.dram_tensor("st4", [NCORES, 4], F32, kind="ExternalInput")
    wd = nc.dram_tensor("wd", [1, F], F32, kind="ExternalInput")
    bnd_g = nc.dram_tensor("bnd_g", [1, F], F32, kind="ExternalInput")
    bnd_b = nc.dram_tensor("bnd_b", [1, F], F32, kind="ExternalInput")
    wg = nc.dram_tensor("wg", [1, F], F32, kind="ExternalInput")
    bng_g = nc.dram_tensor("bng_g", [1, F], F32, kind="ExternalInput")
    bng_b = nc.dram_tensor("bng_b", [1, F], F32, kind="ExternalInput")
    wm = nc.dram_tensor("wm", [3 * F, F], F32, kind="ExternalInput")
    bm = nc.dram_tensor("bm", [1, F], F32, kind="ExternalInput")
    dist_sh = nc.dram_tensor("dist_sh", [TILE, NT], F32, kind="ExternalInput")
    degf_sh = nc.dram_tensor("degf_sh", [TILE, NT], F32, kind="ExternalInput")
    ones_col = nc.dram_tensor("ones_col", [TILE, 1], F32, kind="ExternalInput")
    ones_row = nc.dram_tensor("ones_row", [1, TILE], F32, kind="ExternalInput")
    ident = nc.dram_tensor("ident", [TILE, TILE], F32, kind="ExternalInput")
    out_sh = nc.dram_tensor("out_sh", [RPC, F], F32, kind="ExternalOutput")

    with tile.TileContext(nc) as tc:
        with tc.tile_pool(name="c", bufs=1) as cp, \
             tc.tile_pool(name="w", bufs=3) as wp, \
             tc.tile_pool(name="ps", bufs=2, space="PSUM") as pp, \
             tc.tile_pool(name="pt", bufs=3, space="PSUM") as pt, \
             tc.tile_pool(name="po", bufs=2, space="PSUM") as po:
            ones = cp.tile([TILE, 1], F32)
            nc.sync.dma_start(out=ones[:], in_=ones_col.ap())
            onesr = cp.tile([1, TILE], F32)
            nc.sync.dma_start(out=onesr[:], in_=ones_row.ap())
            idn = cp.tile([TILE, TILE], F32)
            nc.sync.dma_start(out=idn[:], in_=ident.ap())
            g_row = cp.tile([1, F], F32)
            nc.sync.dma_start(out=g_row[:], in_=bn_g.ap())
            b_row = cp.tile([1, F], F32)
            nc.sync.dma_start(out=b_row[:], in_=bn_b.ap())
            dist_t = cp.tile([TILE, NT], F32)
            nc.sync.dma_start(out=dist_t[:], in_=dist_sh.ap())
            degf_t = cp.tile([TILE, NT], F32)
            nc.sync.dma_start(out=degf_t[:], in_=degf_sh.ap())

            s_b, o_b = _bn_finalize(nc, cp, pp, sums, sqs, g_row, b_row,
                                    ones, onesr)

            # scalar-feature stats -> per-feature affine (a, b') columns
            st4_t = cp.tile([NCORES, 4], F32)
            nc.sync.dma_start(out=st4_t[:], in_=st4.ap())
            st_ps = pp.tile([1, 4], F32, space="PSUM", tag="pro")
            nc.tensor.matmul(out=st_ps[:], lhsT=ones[:NCORES, :], rhs=st4_t[:],
                             start=True, stop=True)
            st_row = cp.tile([1, 4], F32)
            nc.vector.tensor_scalar(out=st_row[:], in0=st_ps[:], scalar1=1.0 / N,
                                    scalar2=None, op0=ALU.mult)
            # st_row = (mu_d, E[d^2], mu_g, E[g^2])

            def rank1_cols(w_row_t, g_row_t, b_row_t, mu_ap, m2_ap, tag):
                # a = g * w * rsqrt(var*w^2 + eps); b' = b - mu * a  (rows [1,F])
                var = cp.tile([1, 1], F32, tag=f"{tag}_v")
                nc.vector.tensor_tensor(out=var[:], in0=mu_ap, in1=mu_ap, op=ALU.mult)
                nc.vector.tensor_tensor(out=var[:], in0=m2_ap, in1=var[:],
                                        op=ALU.subtract)
                w2 = cp.tile([1, F], F32, tag=f"{tag}_w2")
                nc.vector.tensor_tensor(out=w2[:], in0=w_row_t[:], in1=w_row_t[:],
                                        op=ALU.mult)
                nc.vector.tensor_scalar(out=w2[:], in0=w2[:], scalar1=var[:],
                                        scalar2=None, op0=ALU.mult)
                nc.vector.tensor_scalar(out=w2[:], in0=w2[:], scalar1=EPS,
                                        scalar2=None, op0=ALU.add)
                sv = cp.tile([1, F], F32, tag=f"{tag}_sv")
                nc.scalar.activation(sv[:], w2[:], ACTF.Sqrt)
                rs = cp.tile([1, F], F32, tag=f"{tag}_rs")
                nc.vector.reciprocal(out=rs[:], in_=sv[:])
                a = cp.tile([1, F], F32, tag=f"{tag}_a")
                nc.vector.tensor_tensor(out=a[:], in0=w_row_t[:], in1=rs[:],
                                        op=ALU.mult)
                nc.vector.tensor_tensor(out=a[:], in0=a[:], in1=g_row_t[:],
                                        op=ALU.mult)
                bp = cp.tile([1, F], F32, tag=f"{tag}_bp")
                nc.vector.tensor_scalar(out=bp[:], in0=a[:], scalar1=mu_ap,
                                        scalar2=None, op0=ALU.mult)
                nc.vector.tensor_tensor(out=bp[:], in0=b_row_t[:], in1=bp[:],
                                        op=ALU.subtract)
                # to columns via matmul with ones[1,1]
                a_ps = pp.tile([TILE, 1], F32, space="PSUM", tag="pro")
                nc.tensor.matmul(out=a_ps[:], lhsT=a[:], rhs=onesr[:, 0:1],
                                 start=True, stop=True)
                a_col = cp.tile([TILE, 1], F32, tag=f"{tag}_ac")
                nc.vector.tensor_copy(out=a_col[:], in_=a_ps[:])
                b_ps = pp.tile([TILE, 1], F32, space="PSUM", tag="pro")
                nc.tensor.matmul(out=b_ps[:], lhsT=bp[:], rhs=onesr[:, 0:1],
                                 start=True, stop=True)
                b_col = cp.tile([TILE, 1], F32, tag=f"{tag}_bc")
                nc.vector.tensor_copy(out=b_col[:], in_=b_ps[:])
                return a_col, b_col

            wd_t = cp.tile([1, F], F32)
            nc.sync.dma_start(out=wd_t[:], in_=wd.ap())
            bndg_t = cp.tile([1, F], F32)
            nc.sync.dma_start(out=bndg_t[:], in_=bnd_g.ap())
            bndb_t = cp.tile([1, F], F32)
            nc.sync.dma_start(out=bndb_t[:], in_=bnd_b.ap())
            wg_t = cp.tile([1, F], F32)
            nc.sync.dma_start(out=wg_t[:], in_=wg.ap())
            bngg_t = cp.tile([1, F], F32)
            nc.sync.dma_start(out=bngg_t[:], in_=bng_g.ap())
            bngb_t = cp.tile([1, F], F32)
            nc.sync.dma_start(out=bngb_t[:], in_=bng_b.ap())

            ad_col, bd_col = rank1_cols(wd_t, bndg_t, bndb_t,
                                        st_row[:, 0:1], st_row[:, 1:2], "d")
            ag_col, bg_col = rank1_cols(wg_t, bngg_t, bngb_t,
                                        st_row[:, 2:3], st_row[:, 3:4], "g")

            wm_bf = []
            for i in range(3):
                w32 = cp.tile([F, F], F32, tag=f"wm{i}_32")
                nc.sync.dma_start(out=w32[:],
                                  in_=wm.ap()[i * F : (i + 1) * F, :])
                wb = cp.tile([F, F], BF16, tag=f"wm{i}_bf")
                nc.vector.tensor_copy(out=wb[:], in_=w32[:])
                wm_bf.append(wb)
            bm_row = cp.tile([1, F], F32)
            nc.sync.dma_start(out=bm_row[:], in_=bm.ap())
            bm_ps = pp.tile([TILE, F], F32, space="PSUM", tag="pro")
            nc.tensor.matmul(out=bm_ps[:], lhsT=onesr[:], rhs=bm_row[:],
                             start=True, stop=True)
            bm_b = cp.tile([TILE, F], F32)
            nc.vector.tensor_copy(out=bm_b[:], in_=bm_ps[:])

            for t in range(NT):
                h2t = wp.tile([TILE, F], F32, tag="h2t")
                nc.sync.dma_start(out=h2t[:],
                                  in_=h2_sh.ap()[t * TILE : (t + 1) * TILE, :])
                t1 = wp.tile([TILE, F], F32, tag="t1")
                nc.vector.tensor_tensor(out=t1[:], in0=h2t[:], in1=s_b[:],
                                        op=ALU.mult)
                nc.vector.tensor_tensor(out=t1[:], in0=t1[:], in1=o_b[:],
                                        op=ALU.add)
                h2n = wp.tile([TILE, F], F32, tag="h2n")
                nc.scalar.activation(h2n[:], t1[:], ACTF.Relu)
                hT_ps = pt.tile([TILE, TILE], F32, space="PSUM", tag="tr")
                nc.tensor.transpose(out=hT_ps[:], in_=h2n[:], identity=idn[:])
                hT = wp.tile([TILE, TILE], BF16, tag="hTb")
                nc.vector.tensor_copy(out=hT[:], in_=hT_ps[:])

                dB_ps = pt.tile([TILE, TILE], F32, space="PSUM", tag="tr")
                nc.tensor.transpose(
                    out=dB_ps[:],
                    in_=dist_t[:, t : t + 1].to_broadcast([TILE, TILE]),
                    identity=idn[:])
                dfT = wp.tile([TILE, TILE], BF16, tag="dfT")
                nc.scalar.activation(dfT[:], dB_ps[:], ACTF.Relu,
                                     scale=ad_col[:], bias=bd_col[:])
                gB_ps = pt.tile([TILE, TILE], F32, space="PSUM", tag="tr")
                nc.tensor.transpose(
                    out=gB_ps[:],
                    in_=degf_t[:, t : t + 1].to_broadcast([TILE, TILE]),
                    identity=idn[:])
                gfT = wp.tile([TILE, TILE], BF16, tag="gfT")
                nc.scalar.activation(gfT[:], gB_ps[:], ACTF.Relu,
                                     scale=ag_col[:], bias=bg_col[:])

                o_ps = po.tile([TILE, F], F32, space="PSUM", tag="o")
                nc.tensor.matmul(out=o_ps[:], lhsT=hT[:], rhs=wm_bf[0][:],
                                 start=True, stop=False)
                nc.tensor.matmul(out=o_ps[:], lhsT=dfT[:], rhs=wm_bf[1][:],
                                 start=False, stop=False)
                nc.tensor.matmul(out=o_ps[:], lhsT=gfT[:], rhs=wm_bf[2][:],
                                 start=False, stop=True)
                ot = wp.tile([TILE, F], F32, tag="ot")
                nc.vector.tensor_tensor(out=ot[:], in0=o_ps[:], in1=bm_b[:],
                                        op=ALU.add)
                nc.sync.dma_start(out=out_sh.ap()[t * TILE : (t + 1) * TILE, :],
                                  in_=ot[:])
    nc.compile()
    return nc


# ----------------------------------------------------------------------------
# cached PJRT SPMD runner (no donation; device-resident inputs; wall timing)
# ----------------------------------------------------------------------------

_RUN_CACHE = {}
LAST_TIMINGS = {}


def _make_runner(nc):
    bass2jax.install_neuronx_cc_hook()
    partition_name = (nc.partition_id_tensor.name
                      if nc.partition_id_tensor else None)
    in_names, out_names, out_avals = [], [], []
    for alloc in nc.m.functions[0].allocations:
        if not isinstance(alloc, mybir.MemoryLocationSet):
            continue
        name = alloc.memorylocations[0].name
        if alloc.kind == "ExternalInput":
            if name != partition_name:
                in_names.append(name)
        elif alloc.kind == "ExternalOutput":
            out_names.append(name)
            out_avals.append(jax.core.ShapedArray(
                tuple(alloc.tensor_shape), mybir.dt.np(alloc.dtype)))
    n_params = len(in_names)
    all_names = in_names + out_names
    if partition_name is not None:
        all_names = all_names + [partition_name]

    def _body(*args):
        operands = list(args)
        if partition_name is not None:
            operands.append(bass2jax.partition_id_tensor())
        outs = bass2jax._bass_exec_p.bind(
            *operands,
            out_avals=tuple(out_avals),
            in_names=tuple(all_names),
            out_names=tuple(out_names),
            lowering_input_output_aliases=(),
            sim_require_finite=True,
            sim_require_nnan=True,
            nc=nc,
        )
        return tuple(outs)

    devices = jax.devices()[:NCORES]
    mesh = Mesh(np.asarray(devices), ("core",))
    sharded = jax.jit(shard_map(
        _body, mesh=mesh,
        in_specs=(PartitionSpec("core"),) * (n_params + len(out_names)),
        out_specs=(PartitionSpec("core"),) * len(out_names),
        check_rep=False))
    return sharded, in_names, out_names, out_avals, mesh


def _run(tag, nc, in_maps, time_it=False):
    key = id(nc)
    if key not in _RUN_CACHE:
        _RUN_CACHE[key] = _make_runner(nc)
    sharded, in_names, out_names, out_avals, mesh = _RUN_CACHE[key]

    concat_in = [
        np.concatenate([np.asarray(in_maps[c][n]) for c in range(NCORES)], axis=0)
        for n in in_names
    ]
    concat_zeros = [
        np.zeros((NCORES * a.shape[0],) + tuple(a.shape[1:]), a.dtype)
        for a in out_avals
    ]
    sh = jax.sharding.NamedSharding(mesh, PartitionSpec("core"))
    dev_in = [jax.device_put(a, sh) for a in concat_in]
    dev_zero = [jax.device_put(a, sh) for a in concat_zeros]
    out = sharded(*dev_in, *dev_zero)
    jax.block_until_ready(out)
    if time_it:
        # marginal per-call time from two pipelined batch sizes -- the first
        # call in a batch carries the RPC/dispatch sync, extra calls queue
        # back-to-back on the device.
        def batch(n):
            t0 = time.perf_counter()
            outs = [sharded(*dev_in, *dev_zero) for _ in range(n)]
            jax.block_until_ready(outs)
            return time.perf_counter() - t0
        batch(2)
        t_small = min(batch(4), batch(4))
        t_big = min(batch(28), batch(28))
        LAST_TIMINGS[tag] = max((t_big - t_small) / 24, 1e-6)
    res = [
        {n: np.asarray(out[i]).reshape((NCORES,) + out_avals[i].shape)[c]
         for i, n in enumerate(out_names)}
        for c in range(NCORES)
    ]
    return res


# ----------------------------------------------------------------------------
# kernel entry point
# ----------------------------------------------------------------------------

_PROG_CACHE = {}


def kernel(x, edge_index, edge_weight, dist_feat, degree_feat,
           W1, b1, W2, b2, bn1_g, bn1_b, bn2_g, bn2_b,
           Wd, bd, bnd_g, bnd_b, Wg, bg, bng_g, bng_b, Wm, bm,
           _time_launches=False):
    edge_index = np.asarray(edge_index)
    new_id = _relabel(edge_index)
    meta, arrays = _prep_edges(edge_index, np.asarray(edge_weight), new_id)

    mkey = (meta["TOTC"], meta["KDTOT"],
            tuple(meta["K_tg"].reshape(-1).tolist()))
    if mkey not in _PROG_CACHE:
        _PROG_CACHE.clear()
        _PROG_CACHE[mkey] = {
            "L0": _build_L0(meta),
            "conv": _build_conv(meta),
            "L2": _build_L2(meta),
            "L4": _build_L4(meta),
        }
    progs = _PROG_CACHE[mkey]

    x = np.asarray(x, np.float32)
    x_sh = _scatter_rows(x, new_id)
    dist_cols = _col_layout(np.asarray(dist_feat)[:, 0], new_id)
    degf_cols = _col_layout(np.asarray(degree_feat)[:, 0], new_id)
    ones_col = np.ones((TILE, 1), np.float32)
    ones_row = np.ones((1, TILE), np.float32)
    ident = np.eye(TILE, dtype=np.float32)
    iota = np.tile(np.arange(TILE, dtype=np.float32).astype(_bf)[None, :],
                   (TILE, 1))

    # ---- L0
    r0 = _run("L0", progs["L0"], [
        {"x_sh": x_sh[c], "ewn": arrays["ewn"][c], "dist_sh": dist_cols[c],
         "degf_sh": degf_cols[c], "ones_col": ones_col}
        for c in range(NCORES)
    ], time_it=_time_launches)
    dinv_own = np.stack([r0[c]["dinv_out"] for c in range(NCORES)])
    xp_full = np.concatenate(
        [r0[c]["xp_out"] for c in range(NCORES)])              # [NV, F] bf16
    st4 = np.stack([r0[c]["st4_out"][0] for c in range(NCORES)])  # [8, 4]

    # ---- L1 (conv1)
    conv_base = [
        {"idx_all": arrays["idx_all"][c], "ew_cols": arrays["ew_cols"][c],
         "dl_cols": arrays["dloc_cols"][c], "dinv": dinv_own[c],
         "iota_in": iota}
        for c in range(NCORES)
    ]
    W1f = np.asarray(W1, np.float32)
    r1 = _run("L1", progs["conv"], [
        dict(m, tbl=xp_full, w_in=W1f) for m in conv_base
    ], time_it=_time_launches)
    h1_sh = [r1[c]["h_out"] for c in range(NCORES)]
    sums1 = np.concatenate([r1[c]["sum_out"] for c in range(NCORES)])
    sqs1 = np.concatenate([r1[c]["sq_out"] for c in range(NCORES)])

    # ---- L2
    r2 = _run("L2", progs["L2"], [
        {"h1_sh": h1_sh[c], "sums": sums1, "sqs": sqs1,
         "bn_g": np.asarray(bn1_g, np.float32)[None, :],
         "bn_b": np.asarray(bn1_b, np.float32)[None, :],
         "dinv": dinv_own[c], "ones_col": ones_col, "ones_row": ones_row}
        for c in range(NCORES)
    ], time_it=_time_launches)
    h1nd_full = np.concatenate(
        [r2[c]["hn_out"] for c in range(NCORES)])              # [NV, F] bf16

    # ---- L3 (conv2, same program)
    W2f = np.asarray(W2, np.float32)
    r3 = _run("L3", progs["conv"], [
        dict(m, tbl=h1nd_full, w_in=W2f) for m in conv_base
    ], time_it=_time_launches)
    h2_sh = [r3[c]["h_out"] for c in range(NCORES)]
    sums2 = np.concatenate([r3[c]["sum_out"] for c in range(NCORES)])
    sqs2 = np.concatenate([r3[c]["sq_out"] for c in range(NCORES)])

    # ---- L4
    r4 = _run("L4", progs["L4"], [
        {"h2_sh": h2_sh[c], "sums": sums2, "sqs": sqs2,
         "bn_g": np.asarray(bn2_g, np.float32)[None, :],
         "bn_b": np.asarray(bn2_b, np.float32)[None, :],
         "st4": st4,
         "wd": np.asarray(Wd, np.float32).reshape(1, F),
         "bnd_g": np.asarray(bnd_g, np.float32)[None, :],
         "bnd_b": np.asarray(bnd_b, np.float32)[None, :],
         "wg": np.asarray(Wg, np.float32).reshape(1, F),
         "bng_g": np.asarray(bng_g, np.float32)[None, :],
         "bng_b": np.asarray(bng_b, np.float32)[None, :],
         "wm": np.asarray(Wm, np.float32), "bm": np.asarray(bm, np.float32)[None, :],
         "dist_sh": dist_cols[c], "degf_sh": degf_cols[c],
         "ones_col": ones_col, "ones_row": ones_row, "ident": ident}
        for c in range(NCORES)
    ], time_it=_time_launches)
    out_nv = np.concatenate([r4[c]["out_sh"] for c in range(NCORES)])
    return out_nv[new_id]


# revision 13
# speedup vs baseline: 1.7454x; 1.7454x over previous
"""GCN encoder (2x GCNConv + BN/ReLU + fused head) on 8 Trainium2 NeuronCores.

Strategy (edge-parallel, dst-owner): each core owns a contiguous range of
output nodes and processes exactly the edges whose destination falls in its
range.  Edges are sorted by destination tile; the per-tile scatter-add is
expressed as a sequence of one-hot matmuls (S_T built on-device with
is_equal against an iota row) accumulated in PSUM.  Source rows are fetched
with the SWDGE dma_gather instruction (int16 indices relative to one of four
25000-row source groups).  Five SPMD launches with host-side concat (layout
only, no host float math on tensor data):

  L0: degree -> dinv per owned node; x' = x * dinv (bf16); dist/deg scalar stats
  L1: conv1: gather x'[src], S-matmul, @W1, *dinv[dst]  -> h1 (f32) + BN1 sums
  L2: BN1 finalize/apply + ReLU + dinv prescale -> h1nd (bf16)
  L3: conv2 (same program as L1): gather h1nd, @W2 -> h2 (f32) + BN2 sums
  L4: BN2 apply + ReLU + head (h2n@Wm_h + rank-1 dist/deg branches + bm)
"""

import time

import numpy as np
import ml_dtypes

import jax
import jax.numpy as jnp
from jax.sharding import Mesh, PartitionSpec
from jax.experimental.shard_map import shard_map

from concourse import bacc, mybir
import concourse.bass as bass
import concourse.tile as tile
from concourse import bass2jax
from concourse.library_config import mlp

F32 = mybir.dt.float32
BF16 = mybir.dt.bfloat16
I16 = mybir.dt.int16
ALU = mybir.AluOpType
ACTF = mybir.ActivationFunctionType

N = 100000
E = 1600000
F = 128
NCORES = 8
RPC = 12544          # rows per core (98 tiles of 128); core 7 real rows: 12192
NT = 98              # dst tiles per core
TILE = 128
GROUPS = 4           # int16 src index groups
NV = NCORES * RPC   # padded node-id space (relabeled)
GSZ = NV // GROUPS   # 25088 < 32768 (int16 ok)
TB = 8               # dst tiles per gather block
EPS = 1e-5
MAX_GCHUNK = 60      # max chunks per dma_gather instruction (<= 8192 idx limit)

BLOCKS = [list(range(b, min(b + TB, NT))) for b in range(0, NT, TB)]

_bf = ml_dtypes.bfloat16

# ----------------------------------------------------------------------------
# host-side index prep (layout / sorting / padding only -- no tensor math)
# ----------------------------------------------------------------------------


def _relabel(edge_index):
    """degree-balanced node permutation: heavy nodes spread round-robin over
    the 8*98 global tiles (snake order).  Returns new_id[old] in [0, NV)."""
    indeg = np.bincount(edge_index[1].astype(np.int64), minlength=N)
    order = np.argsort(-indeg, kind="stable")
    NTG = NCORES * NT
    pos = np.arange(N)
    rnd = pos // NTG
    tir = pos % NTG
    tilei = np.where(rnd % 2 == 0, tir, NTG - 1 - tir)
    new_global = (tilei // NT) * RPC + (tilei % NT) * TILE + rnd
    new_id = np.empty(N, np.int64)
    new_id[order] = new_global
    return new_id


def _prep_edges(edge_index, edge_weight, new_id):
    src = new_id[edge_index[0].astype(np.int64)]
    dst = new_id[edge_index[1].astype(np.int64)]
    loops = new_id.copy()
    src2 = np.concatenate([src, loops])
    dst2 = np.concatenate([dst, loops])
    ew2 = np.concatenate(
        [edge_weight.astype(np.float32), np.ones(N, np.float32)]
    )
    M = src2.shape[0]

    core = dst2 // RPC
    rloc = dst2 - core * RPC
    t = rloc // TILE
    dloc = (rloc % TILE).astype(np.float32)
    g = src2 // GSZ
    src_rel = (src2 - g * GSZ).astype(np.int16)

    seg = (core * NT + t) * GROUPS + g           # global segment id
    order = np.argsort(seg, kind="stable")
    seg_s = seg[order]
    counts = np.bincount(seg, minlength=NCORES * NT * GROUPS)
    counts_ctg = counts.reshape(NCORES, NT, GROUPS)

    # compile-time chunk map: shared by all cores
    K_tg = -(-counts_ctg.max(axis=0) // TILE)     # [NT, GROUPS] ceil
    for ti in range(NT):
        if K_tg[ti].sum() == 0:
            K_tg[ti][0] = 1
    TOTC = int(K_tg.sum())

    Kflat = K_tg.reshape(-1)                      # [NT*GROUPS] in (t, g) order
    chunk_off = np.concatenate([[0], np.cumsum(Kflat)])[:-1].reshape(NT, GROUPS)

    # slot of each edge: core*TOTC*128 + chunk_off[t,g]*128 + rank_in_segment
    starts = np.concatenate([[0], np.cumsum(counts)])[:-1]
    rank = np.arange(M) - starts[seg_s]
    tg_s = seg_s % (NT * GROUPS)
    slot = seg_s // (NT * GROUPS) * (TOTC * TILE) + chunk_off.reshape(-1)[tg_s] * TILE + rank

    src_slots = np.zeros(NCORES * TOTC * TILE, np.int16)
    ew_slots = np.zeros(NCORES * TOTC * TILE, np.float32)
    dloc_slots = np.zeros(NCORES * TOTC * TILE, np.float32)
    src_slots[slot] = src_rel[order]
    ew_slots[slot] = ew2[order]
    dloc_slots[slot] = dloc[order]
    src_slots = src_slots.reshape(NCORES, TOTC, TILE)
    ew_slots = ew_slots.reshape(NCORES, TOTC, TILE)
    dloc_slots = dloc_slots.reshape(NCORES, TOTC, TILE)

    # meta columns in (t, g, k) order: [cores, 128, TOTC]
    ew_cols = np.ascontiguousarray(np.swapaxes(ew_slots, 1, 2))
    dloc_cols = np.ascontiguousarray(np.swapaxes(dloc_slots, 1, 2))

    # gather chunk lists per (block, g): chunks of tiles in block, split to
    # pieces of <= MAX_GCHUNK chunks
    gather_plan = []      # list over blocks of list over g of list of pieces
    idx_parts = []        # int16 wrapped index arrays, per piece, per core
    for blk in BLOCKS:
        per_g = []
        for gi in range(GROUPS):
            chunk_ids = []
            for ti in blk:
                chunk_ids.extend(
                    range(chunk_off[ti, gi], chunk_off[ti, gi] + K_tg[ti, gi])
                )
            pieces = [
                chunk_ids[i : i + MAX_GCHUNK]
                for i in range(0, len(chunk_ids), MAX_GCHUNK)
            ]
            per_g.append(pieces)
            for piece in pieces:
                flat = src_slots[:, piece, :].reshape(NCORES, -1)  # [8, n*128]
                wrapped = np.tile(
                    flat.reshape(NCORES, -1, 16).swapaxes(1, 2), (1, 8, 1)
                )  # [8, 128, n*8]
                idx_parts.append(wrapped)
        gather_plan.append(per_g)
    idx_all = np.concatenate(idx_parts, axis=2)   # [8, 128, TOTC*8]

    # per-node edge-weight table for degree computation
    full_counts = np.bincount(dst2, minlength=NV)
    ctile = full_counts.reshape(NCORES, NT, TILE)
    KD_t = ctile.max(axis=(0, 2))
    KD_t = np.maximum(KD_t, 1)
    KDoff = np.concatenate([[0], np.cumsum(KD_t)])[:-1]
    KDTOT = int(KD_t.sum())

    o2 = np.argsort(dst2, kind="stable")
    dst_s2 = dst2[o2]
    st2 = np.concatenate([[0], np.cumsum(full_counts)])[:-1]
    rank2 = np.arange(M) - st2[dst_s2]
    c2 = dst_s2 // RPC
    rl2 = dst_s2 - c2 * RPC
    t2 = rl2 // TILE
    p2 = rl2 % TILE
    ewn = np.zeros((NCORES, TILE, KDTOT), np.float32)
    ewn[c2, p2, KDoff[t2] + rank2] = ew2[o2]

    meta = {
        "K_tg": K_tg,
        "chunk_off": chunk_off,
        "TOTC": TOTC,
        "gather_plan": gather_plan,
        "KD_t": KD_t,
        "KDoff": KDoff,
        "KDTOT": KDTOT,
    }
    arrays = {
        "ew_cols": ew_cols,
        "dloc_cols": dloc_cols,
        "idx_all": idx_all,
        "ewn": ewn,
    }
    return meta, arrays


def _scatter_rows(a, new_id):
    """[N, ...] -> [8, RPC, ...]: row old-i lands at new_id[i]."""
    out = np.zeros((NV,) + a.shape[1:], a.dtype)
    out[new_id] = a
    return out.reshape((NCORES, RPC) + a.shape[1:])


def _col_layout(a, new_id):
    """[N] -> [8, 128, NT]  with relabeled node t*128+p at [c, p, t]."""
    padded = np.zeros(NV, np.float32)
    padded[new_id] = a.astype(np.float32)
    return np.ascontiguousarray(
        padded.reshape(NCORES, NT, TILE).swapaxes(1, 2)
    )


# ----------------------------------------------------------------------------
# bass program builders
# ----------------------------------------------------------------------------


def _new_nc():
    return bacc.Bacc("TRN2", target_bir_lowering=False, debug=False,
                     num_devices=NCORES)


def _build_L0(meta):
    KD_t, KDoff, KDTOT = meta["KD_t"], meta["KDoff"], meta["KDTOT"]
    nc = _new_nc()
    x_sh = nc.dram_tensor("x_sh", [RPC, F], F32, kind="ExternalInput")
    ewn = nc.dram_tensor("ewn", [TILE, KDTOT], F32, kind="ExternalInput")
    dist_sh = nc.dram_tensor("dist_sh", [TILE, NT], F32, kind="ExternalInput")
    degf_sh = nc.dram_tensor("degf_sh", [TILE, NT], F32, kind="ExternalInput")
    ones_col = nc.dram_tensor("ones_col", [TILE, 1], F32, kind="ExternalInput")
    dinv_out = nc.dram_tensor("dinv_out", [TILE, NT], F32, kind="ExternalOutput")
    xp_out = nc.dram_tensor("xp_out", [RPC, F], BF16, kind="ExternalOutput")
    st4_out = nc.dram_tensor("st4_out", [1, 4], F32, kind="ExternalOutput")

    with tile.TileContext(nc) as tc:
        with tc.tile_pool(name="sb", bufs=1) as cp, \
             tc.tile_pool(name="work", bufs=3) as wp, \
             tc.tile_pool(name="ps", bufs=2, space="PSUM") as pp:
            ewt = cp.tile([TILE, KDTOT], F32)
            nc.sync.dma_start(out=ewt[:], in_=ewn.ap())
            ones = cp.tile([TILE, 1], F32)
            nc.sync.dma_start(out=ones[:], in_=ones_col.ap())
            dist_t = cp.tile([TILE, NT], F32)
            nc.sync.dma_start(out=dist_t[:], in_=dist_sh.ap())
            degf_t = cp.tile([TILE, NT], F32)
            nc.sync.dma_start(out=degf_t[:], in_=degf_sh.ap())
            dinv_sb = cp.tile([TILE, NT], F32)
            deg_sb = cp.tile([TILE, NT], F32)

            for t in range(NT):
                nc.vector.tensor_reduce(
                    out=deg_sb[:, t : t + 1],
                    in_=ewt[:, int(KDoff[t]) : int(KDoff[t] + KD_t[t])],
                    axis=mybir.AxisListType.X, op=ALU.add)
            m0 = cp.tile([TILE, NT], F32)
            nc.vector.tensor_scalar(out=m0[:], in0=deg_sb[:], scalar1=0.0,
                                    scalar2=None, op0=ALU.is_equal)
            nc.vector.tensor_tensor(out=deg_sb[:], in0=deg_sb[:], in1=m0[:],
                                    op=ALU.add)
            sqd = cp.tile([TILE, NT], F32)
            nc.scalar.activation(sqd[:], deg_sb[:], ACTF.Sqrt)
            nc.vector.reciprocal(out=dinv_sb[:], in_=sqd[:])

            XB = 7  # row-tiles per DMA (98 = 14*7)
            for tb in range(0, NT, XB):
                xt = wp.tile([TILE, XB * F], F32, tag="xt")
                nc.sync.dma_start(
                    out=xt[:].rearrange("p (j f) -> p j f", f=F),
                    in_=x_sh.ap()[tb * TILE : (tb + XB) * TILE, :].rearrange(
                        "(j p) f -> p j f", p=TILE))
                xp = wp.tile([TILE, XB * F], BF16, tag="xp")
                for j in range(XB):
                    nc.scalar.activation(
                        xp[:, j * F : (j + 1) * F], xt[:, j * F : (j + 1) * F],
                        ACTF.Copy, scale=dinv_sb[:, tb + j : tb + j + 1])
                nc.sync.dma_start(
                    out=xp_out.ap()[tb * TILE : (tb + XB) * TILE, :].rearrange(
                        "(j p) f -> p j f", p=TILE),
                    in_=xp[:].rearrange("p (j f) -> p j f", f=F))

            nc.sync.dma_start(out=dinv_out.ap(), in_=dinv_sb[:])

            # scalar-feature stats: columns (sum_d, sumsq_d, sum_g, sumsq_g)
            scols = cp.tile([TILE, 4], F32)
            nc.vector.tensor_reduce(out=scols[:, 0:1], in_=dist_t[:],
                                    axis=mybir.AxisListType.X, op=ALU.add)
            d2 = cp.tile([TILE, NT], F32)
            nc.scalar.activation(d2[:], dist_t[:], ACTF.Square)
            nc.vector.tensor_reduce(out=scols[:, 1:2], in_=d2[:],
                                    axis=mybir.AxisListType.X, op=ALU.add)
            nc.vector.tensor_reduce(out=scols[:, 2:3], in_=degf_t[:],
                                    axis=mybir.AxisListType.X, op=ALU.add)
            g2 = cp.tile([TILE, NT], F32)
            nc.scalar.activation(g2[:], degf_t[:], ACTF.Square)
            nc.vector.tensor_reduce(out=scols[:, 3:4], in_=g2[:],
                                    axis=mybir.AxisListType.X, op=ALU.add)
            sps = pp.tile([1, 4], F32, space="PSUM")
            nc.tensor.matmul(out=sps[:], lhsT=ones[:], rhs=scols[:],
                             start=True, stop=True)
            srow = cp.tile([1, 4], F32)
            nc.vector.tensor_copy(out=srow[:], in_=sps[:])
            nc.sync.dma_start(out=st4_out.ap(), in_=srow[:])
    nc.compile()
    return nc


def _build_conv(meta):
    """Shared program for conv1 (tbl=x', W=W1) and conv2 (tbl=h1nd, W=W2)."""
    K_tg, chunk_off, TOTC = meta["K_tg"], meta["chunk_off"], meta["TOTC"]
    gather_plan = meta["gather_plan"]

    nc = _new_nc()
    tbl = nc.dram_tensor("tbl", [NV, F], BF16, kind="ExternalInput")
    idx_all = nc.dram_tensor("idx_all", [TILE, TOTC * 8], I16, kind="ExternalInput")
    ew_cols = nc.dram_tensor("ew_cols", [TILE, TOTC], F32, kind="ExternalInput")
    dl_cols = nc.dram_tensor("dl_cols", [TILE, TOTC], F32, kind="ExternalInput")
    dinv = nc.dram_tensor("dinv", [TILE, NT], F32, kind="ExternalInput")
    w_in = nc.dram_tensor("w_in", [F, F], F32, kind="ExternalInput")
    iota_in = nc.dram_tensor("iota_in", [TILE, TILE], BF16, kind="ExternalInput")
    h_out = nc.dram_tensor("h_out", [RPC, F], F32, kind="ExternalOutput")
    sum_out = nc.dram_tensor("sum_out", [TILE, F], F32, kind="ExternalOutput")
    sq_out = nc.dram_tensor("sq_out", [TILE, F], F32, kind="ExternalOutput")

    with tile.TileContext(nc) as tc:
        nc.gpsimd.load_library(mlp)
        with tc.tile_pool(name="const", bufs=1) as cp, \
             tc.tile_pool(name="gat", bufs=2) as gp, \
             tc.tile_pool(name="meta", bufs=2) as mp, \
             tc.tile_pool(name="work", bufs=6) as wp, \
             tc.tile_pool(name="acc", bufs=4, space="PSUM") as ap, \
             tc.tile_pool(name="hp", bufs=4, space="PSUM") as hp:
            iota_t = cp.tile([TILE, TILE], BF16)
            nc.sync.dma_start(out=iota_t[:], in_=iota_in.ap())
            w32 = cp.tile([F, F], F32)
            nc.sync.dma_start(out=w32[:], in_=w_in.ap())
            wbf = cp.tile([F, F], BF16)
            nc.vector.tensor_copy(out=wbf[:], in_=w32[:])
            dinv_t = cp.tile([TILE, NT], F32)
            nc.sync.dma_start(out=dinv_t[:], in_=dinv.ap())
            sum_acc = cp.tile([TILE, F], F32)
            nc.vector.memset(sum_acc[:], 0.0)
            sq_acc = cp.tile([TILE, F], F32)
            nc.vector.memset(sq_acc[:], 0.0)

            goff = 0  # running chunk offset inside idx_all
            for bi, blk in enumerate(BLOCKS):
                # gathers for this block, one tile buffer per group
                gts = []
                gpos0 = []  # start chunk (within group buffer) per tile
                for gi in range(GROUPS):
                    pieces = gather_plan[bi][gi]
                    nch = sum(len(p) for p in pieces)
                    if nch == 0:
                        gts.append(None)
                        gpos0.append(None)
                        continue
                    gt = gp.tile([TILE, nch * TILE], BF16, tag=f"g{gi}")
                    pos = 0
                    for piece in pieces:
                        npc = len(piece)
                        it = mp.tile([TILE, npc * 8], I16, tag=f"i{gi}")
                        nc.sync.dma_start(
                            out=it[:],
                            in_=idx_all.ap()[:, goff * 8 : (goff + npc) * 8])
                        base = gi * GSZ
                        top = base + GSZ
                        out_ap = gt[:, pos * F : (pos + npc) * F].rearrange(
                            "p (c d) -> p c d", d=F)
                        nc.gpsimd.dma_gather(
                            out_ap, tbl.ap()[base:top, :], it[:],
                            npc * TILE, npc * TILE, F,
                            single_packet=False,
                        )
                        pos += npc
                        goff += npc
                    gts.append(gt)
                    starts = {}
                    s = 0
                    for ti in blk:
                        starts[ti] = s
                        s += int(K_tg[ti, gi])
                    gpos0.append(starts)

                c0 = int(chunk_off[blk[0], 0])
                cb = int(K_tg[blk, :].sum())
                ewt = mp.tile([TILE, cb], F32, tag="ew")
                nc.sync.dma_start(out=ewt[:], in_=ew_cols.ap()[:, c0 : c0 + cb])
                dlt = mp.tile([TILE, cb], F32, tag="dl")
                nc.sync.dma_start(out=dlt[:], in_=dl_cols.ap()[:, c0 : c0 + cb])

                for ti in blk:
                    ntc = int(K_tg[ti].sum())
                    acc = ap.tile([TILE, TILE], F32, space="PSUM", tag="acc")
                    j = 0
                    for gi in range(GROUPS):
                        kk = int(K_tg[ti, gi])
                        for k in range(kk):
                            col = int(chunk_off[ti, gi]) + k - c0
                            st = wp.tile([TILE, TILE], BF16, tag="st")
                            nc.vector.tensor_scalar(
                                out=st[:], in0=iota_t[:],
                                scalar1=dlt[:, col : col + 1],
                                scalar2=ewt[:, col : col + 1],
                                op0=ALU.is_equal, op1=ALU.mult)
                            gslice = gts[gi][:, (gpos0[gi][ti] + k) * F
                                             : (gpos0[gi][ti] + k + 1) * F]
                            nc.tensor.matmul(out=acc[:], lhsT=gslice, rhs=st[:],
                                             start=(j == 0), stop=(j == ntc - 1))
                            j += 1
                    accs = wp.tile([TILE, TILE], BF16, tag="accs")
                    nc.vector.tensor_copy(out=accs[:], in_=acc[:])
                    h_ps = hp.tile([TILE, F], F32, space="PSUM", tag="h")
                    nc.tensor.matmul(out=h_ps[:], lhsT=accs[:], rhs=wbf[:],
                                     start=True, stop=True)
                    h32 = wp.tile([TILE, F], F32, tag="h32")
                    nc.vector.tensor_scalar(
                        out=h32[:], in0=h_ps[:],
                        scalar1=dinv_t[:, ti : ti + 1], scalar2=None,
                        op0=ALU.mult)
                    nc.vector.tensor_tensor(out=sum_acc[:], in0=sum_acc[:],
                                            in1=h32[:], op=ALU.add)
                    hsq = wp.tile([TILE, F], F32, tag="hsq")
                    nc.scalar.activation(hsq[:], h32[:], ACTF.Square)
                    nc.vector.tensor_tensor(out=sq_acc[:], in0=sq_acc[:],
                                            in1=hsq[:], op=ALU.add)
                    nc.sync.dma_start(
                        out=h_out.ap()[ti * TILE : (ti + 1) * TILE, :],
                        in_=h32[:])

            nc.sync.dma_start(out=sum_out.ap(), in_=sum_acc[:])
            nc.sync.dma_start(out=sq_out.ap(), in_=sq_acc[:])
    nc.compile()
    return nc


def _bn_finalize(nc, cp, pp, sums_t, sqs_t, g_row, b_row, ones, ones_row):
    """device-side BN scale/offset from stacked per-core partial sums.

    Returns (s_b, o_b): [128,128] broadcast tiles (f32, SBUF).
    sums_t/sqs_t: input DRAM tensors [8*128, 128].
    """
    tot_s = cp.tile([TILE, F], F32, tag="bn_ts")
    tot_q = cp.tile([TILE, F], F32, tag="bn_tq")
    for i in range(NCORES):
        a = cp.tile([TILE, F], F32, tag="bn_a")
        nc.sync.dma_start(out=a[:], in_=sums_t.ap()[i * TILE : (i + 1) * TILE, :])
        if i == 0:
            nc.vector.tensor_copy(out=tot_s[:], in_=a[:])
        else:
            nc.vector.tensor_tensor(out=tot_s[:], in0=tot_s[:], in1=a[:], op=ALU.add)
        b = cp.tile([TILE, F], F32, tag="bn_b")
        nc.sync.dma_start(out=b[:], in_=sqs_t.ap()[i * TILE : (i + 1) * TILE, :])
        if i == 0:
            nc.vector.tensor_copy(out=tot_q[:], in_=b[:])
        else:
            nc.vector.tensor_tensor(out=tot_q[:], in0=tot_q[:], in1=b[:], op=ALU.add)
    cs = pp.tile([1, F], F32, space="PSUM", tag="pro")
    nc.tensor.matmul(out=cs[:], lhsT=ones[:], rhs=tot_s[:], start=True, stop=True)
    mu = cp.tile([1, F], F32, tag="bn_mu")
    nc.vector.tensor_scalar(out=mu[:], in0=cs[:], scalar1=1.0 / N, scalar2=None,
                            op0=ALU.mult)
    cq = pp.tile([1, F], F32, space="PSUM", tag="pro")
    nc.tensor.matmul(out=cq[:], lhsT=ones[:], rhs=tot_q[:], start=True, stop=True)
    msq = cp.tile([1, F], F32, tag="bn_msq")
    nc.vector.tensor_scalar(out=msq[:], in0=cq[:], scalar1=1.0 / N, scalar2=None,
                            op0=ALU.mult)
    var = cp.tile([1, F], F32, tag="bn_var")
    nc.vector.tensor_tensor(out=var[:], in0=mu[:], in1=mu[:], op=ALU.mult)
    nc.vector.tensor_tensor(out=var[:], in0=msq[:], in1=var[:], op=ALU.subtract)
    nc.vector.tensor_scalar(out=var[:], in0=var[:], scalar1=EPS, scalar2=None,
                            op0=ALU.add)
    sv = cp.tile([1, F], F32, tag="bn_sv")
    nc.scalar.activation(sv[:], var[:], ACTF.Sqrt)
    rs = cp.tile([1, F], F32, tag="bn_rs")
    nc.vector.reciprocal(out=rs[:], in_=sv[:])
    s1 = cp.tile([1, F], F32, tag="bn_s1")
    nc.vector.tensor_tensor(out=s1[:], in0=g_row[:], in1=rs[:], op=ALU.mult)
    o1 = cp.tile([1, F], F32, tag="bn_o1")
    nc.vector.tensor_tensor(out=o1[:], in0=mu[:], in1=s1[:], op=ALU.mult)
    nc.vector.tensor_tensor(out=o1[:], in0=b_row[:], in1=o1[:], op=ALU.subtract)
    sb_ps = pp.tile([TILE, F], F32, space="PSUM", tag="pro")
    nc.tensor.matmul(out=sb_ps[:], lhsT=ones_row[:], rhs=s1[:], start=True, stop=True)
    s_b = cp.tile([TILE, F], F32, tag="bn_sb")
    nc.vector.tensor_copy(out=s_b[:], in_=sb_ps[:])
    ob_ps = pp.tile([TILE, F], F32, space="PSUM", tag="pro")
    nc.tensor.matmul(out=ob_ps[:], lhsT=ones_row[:], rhs=o1[:], start=True, stop=True)
    o_b = cp.tile([TILE, F], F32, tag="bn_ob")
    nc.vector.tensor_copy(out=o_b[:], in_=ob_ps[:])
    return s_b, o_b


def _build_L2(meta):
    nc = _new_nc()
    h1_sh = nc.dram_tensor("h1_sh", [RPC, F], F32, kind="ExternalInput")
    sums = nc.dram_tensor("sums", [NCORES * TILE, F], F32, kind="ExternalInput")
    sqs = nc.dram_tensor("sqs", [NCORES * TILE, F], F32, kind="ExternalInput")
    bn_g = nc.dram_tensor("bn_g", [1, F], F32, kind="ExternalInput")
    bn_b = nc.dram_tensor("bn_b", [1, F], F32, kind="ExternalInput")
    dinv = nc.dram_tensor("dinv", [TILE, NT], F32, kind="ExternalInput")
    ones_col = nc.dram_tensor("ones_col", [TILE, 1], F32, kind="ExternalInput")
    ones_row = nc.dram_tensor("ones_row", [1, TILE], F32, kind="ExternalInput")
    hn_out = nc.dram_tensor("hn_out", [RPC, F], BF16, kind="ExternalOutput")

    with tile.TileContext(nc) as tc:
        with tc.tile_pool(name="c", bufs=1) as cp, \
             tc.tile_pool(name="w", bufs=3) as wp, \
             tc.tile_pool(name="ps", bufs=2, space="PSUM") as pp:
            ones = cp.tile([TILE, 1], F32)
            nc.sync.dma_start(out=ones[:], in_=ones_col.ap())
            onesr = cp.tile([1, TILE], F32)
            nc.sync.dma_start(out=onesr[:], in_=ones_row.ap())
            g_row = cp.tile([1, F], F32)
            nc.sync.dma_start(out=g_row[:], in_=bn_g.ap())
            b_row = cp.tile([1, F], F32)
            nc.sync.dma_start(out=b_row[:], in_=bn_b.ap())
            dinv_t = cp.tile([TILE, NT], F32)
            nc.sync.dma_start(out=dinv_t[:], in_=dinv.ap())

            s_b, o_b = _bn_finalize(nc, cp, pp, sums, sqs, g_row, b_row,
                                    ones, onesr)

            XB = 7
            for tb in range(0, NT, XB):
                ht = wp.tile([TILE, XB * F], F32, tag="ht")
                nc.sync.dma_start(
                    out=ht[:].rearrange("p (j f) -> p j f", f=F),
                    in_=h1_sh.ap()[tb * TILE : (tb + XB) * TILE, :].rearrange(
                        "(j p) f -> p j f", p=TILE))
                hn = wp.tile([TILE, XB * F], BF16, tag="hn")
                for j in range(XB):
                    t1 = wp.tile([TILE, F], F32, tag="t1")
                    nc.vector.tensor_tensor(
                        out=t1[:], in0=ht[:, j * F : (j + 1) * F], in1=s_b[:],
                        op=ALU.mult)
                    nc.vector.tensor_tensor(out=t1[:], in0=t1[:], in1=o_b[:],
                                            op=ALU.add)
                    nc.scalar.activation(
                        hn[:, j * F : (j + 1) * F], t1[:], ACTF.Relu,
                        scale=dinv_t[:, tb + j : tb + j + 1])
                nc.sync.dma_start(
                    out=hn_out.ap()[tb * TILE : (tb + XB) * TILE, :].rearrange(
                        "(j p) f -> p j f", p=TILE),
                    in_=hn[:].rearrange("p (j f) -> p j f", f=F))
    nc.compile()
    return nc


def _build_L4(meta):
    nc = _new_nc()
    h2_sh = nc.dram_tensor("h2_sh", [RPC, F], F32, kind="ExternalInput")
    sums = nc.dram_tensor("sums", [NCORES * TILE, F], F32, kind="ExternalInput")
    sqs = nc.dram_tensor("sqs", [NCORES * TILE, F], F32, kind="ExternalInput")
    bn_g = nc.dram_tensor("bn_g", [1, F], F32, kind="ExternalInput")
    bn_b = nc.dram_tensor("bn_b", [1, F], F32, kind="ExternalInput")
    st4 = nc.dram_tensor("st4", [NCORES, 4], F32, kind="ExternalInput")
    wd = nc.dram_tensor("wd", [1, F], F32, kind="ExternalInput")
    bnd_g = nc.dram_tensor("bnd_g", [1, F], F32, kind="ExternalInput")
    bnd_b = nc.dram_tensor("bnd_b", [1, F], F32, kind="ExternalInput")
    wg = nc.dram_tensor("wg", [1, F], F32, kind="ExternalInput")
    bng_g = nc.dram_tensor("bng_g", [1, F], F32, kind="ExternalInput")
    bng_b = nc.dram_tensor("bng_b", [1, F], F32, kind="ExternalInput")
    wm = nc.dram_tensor("wm", [3 * F, F], F32, kind="ExternalInput")
    bm = nc.dram_tensor("bm", [1, F], F32, kind="ExternalInput")
    dist_sh = nc.dram_tensor("dist_sh", [TILE, NT], F32, kind="ExternalInput")
    degf_sh = nc.dram_tensor("degf_sh", [TILE, NT], F32, kind="ExternalInput")
    ones_col = nc.dram_tensor("ones_col", [TILE, 1], F32, kind="ExternalInput")
    ones_row = nc.dram_tensor("ones_row", [1, TILE], F32, kind="ExternalInput")
    ident = nc.dram_tensor("ident", [TILE, TILE], F32, kind="ExternalInput")
    out_sh = nc.dram_tensor("out_sh", [RPC, F], F32, kind="ExternalOutput")

    with tile.TileContext(nc) as tc:
        with tc.tile_pool(name="c", bufs=1) as cp, \
             tc.tile_pool(name="w", bufs=3) as wp, \
             tc.tile_pool(name="ps", bufs=2, space="PSUM") as pp, \
             tc.tile_pool(name="pt", bufs=3, space="PSUM") as pt, \
             tc.tile_pool(name="po", bufs=2, space="PSUM") as po:
            ones = cp.tile([TILE, 1], F32)
            nc.sync.dma_start(out=ones[:], in_=ones_col.ap())
            onesr = cp.tile([1, TILE], F32)
            nc.sync.dma_start(out=onesr[:], in_=ones_row.ap())
            idn = cp.tile([TILE, TILE], F32)
            nc.sync.dma_start(out=idn[:], in_=ident.ap())
            g_row = cp.tile([1, F], F32)
            nc.sync.dma_start(out=g_row[:], in_=bn_g.ap())
            b_row = cp.tile([1, F], F32)
            nc.sync.dma_start(out=b_row[:], in_=bn_b.ap())
            dist_t = cp.tile([TILE, NT], F32)
            nc.sync.dma_start(out=dist_t[:], in_=dist_sh.ap())
            degf_t = cp.tile([TILE, NT], F32)
            nc.sync.dma_start(out=degf_t[:], in_=degf_sh.ap())

            s_b, o_b = _bn_finalize(nc, cp, pp, sums, sqs, g_row, b_row,
                                    ones, onesr)

            # scalar-feature stats -> per-feature affine (a, b') columns
            st4_t = cp.tile([NCORES, 4], F32)
            nc.sync.dma_start(out=st4_t[:], in_=st4.ap())
            st_ps = pp.tile([1, 4], F32, space="PSUM", tag="pro")
            nc.tensor.matmul(out=st_ps[:], lhsT=ones[:NCORES, :], rhs=st4_t[:],
                             start=True, stop=True)
            st_row = cp.tile([1, 4], F32)
            nc.vector.tensor_scalar(out=st_row[:], in0=st_ps[:], scalar1=1.0 / N,
                                    scalar2=None, op0=ALU.mult)
            # st_row = (mu_d, E[d^2], mu_g, E[g^2])

            def rank1_cols(w_row_t, g_row_t, b_row_t, mu_ap, m2_ap, tag):
                # a = g * w * rsqrt(var*w^2 + eps); b' = b - mu * a  (rows [1,F])
                var = cp.tile([1, 1], F32, tag=f"{tag}_v")
                nc.vector.tensor_tensor(out=var[:], in0=mu_ap, in1=mu_ap, op=ALU.mult)
                nc.vector.tensor_tensor(out=var[:], in0=m2_ap, in1=var[:],
                                        op=ALU.subtract)
                w2 = cp.tile([1, F], F32, tag=f"{tag}_w2")
                nc.vector.tensor_tensor(out=w2[:], in0=w_row_t[:], in1=w_row_t[:],
                                        op=ALU.mult)
                nc.vector.tensor_scalar(out=w2[:], in0=w2[:], scalar1=var[:],
                                        scalar2=None, op0=ALU.mult)
                nc.vector.tensor_scalar(out=w2[:], in0=w2[:], scalar1=EPS,
                                        scalar2=None, op0=ALU.add)
                sv = cp.tile([1, F], F32, tag=f"{tag}_sv")
                nc.scalar.activation(sv[:], w2[:], ACTF.Sqrt)
                rs = cp.tile([1, F], F32, tag=f"{tag}_rs")
                nc.vector.reciprocal(out=rs[:], in_=sv[:])
                a = cp.tile([1, F], F32, tag=f"{tag}_a")
                nc.vector.tensor_tensor(out=a[:], in0=w_row_t[:], in1=rs[:],
                                        op=ALU.mult)
                nc.vector.tensor_tensor(out=a[:], in0=a[:], in1=g_row_t[:],
                                        op=ALU.mult)
                bp = cp.tile([1, F], F32, tag=f"{tag}_bp")
                nc.vector.tensor_scalar(out=bp[:], in0=a[:], scalar1=mu_ap,
                                        scalar2=None, op0=ALU.mult)
                nc.vector.tensor_tensor(out=bp[:], in0=b_row_t[:], in1=bp[:],
                                        op=ALU.subtract)
                # to columns via matmul with ones[1,1]
                a_ps = pp.tile([TILE, 1], F32, space="PSUM", tag="pro")
                nc.tensor.matmul(out=a_ps[:], lhsT=a[:], rhs=onesr[:, 0:1],
                                 start=True, stop=True)
                a_col = cp.tile([TILE, 1], F32, tag=f"{tag}_ac")
                nc.vector.tensor_copy(out=a_col[:], in_=a_ps[:])
                b_ps = pp.tile([TILE, 1], F32, space="PSUM", tag="pro")
                nc.tensor.matmul(out=b_ps[:], lhsT=bp[:], rhs=onesr[:, 0:1],
                                 start=True, stop=True)
                b_col = cp.tile([TILE, 1], F32, tag=f"{tag}_bc")
                nc.vector.tensor_copy(out=b_col[:], in_=b_ps[:])
                return a_col, b_col

            wd_t = cp.tile([1, F], F32)
            nc.sync.dma_start(out=wd_t[:], in_=wd.ap())
            bndg_t = cp.tile([1, F], F32)
            nc.sync.dma_start(out=bndg_t[:], in_=bnd_g.ap())
            bndb_t = cp.tile([1, F], F32)
            nc.sync.dma_start(out=bndb_t[:], in_=bnd_b.ap())
            wg_t = cp.tile([1, F], F32)
            nc.sync.dma_start(out=wg_t[:], in_=wg.ap())
            bngg_t = cp.tile([1, F], F32)
            nc.sync.dma_start(out=bngg_t[:], in_=bng_g.ap())
            bngb_t = cp.tile([1, F], F32)
            nc.sync.dma_start(out=bngb_t[:], in_=bng_b.ap())

            ad_col, bd_col = rank1_cols(wd_t, bndg_t, bndb_t,
                                        st_row[:, 0:1], st_row[:, 1:2], "d")
            ag_col, bg_col = rank1_cols(wg_t, bngg_t, bngb_t,
                                        st_row[:, 2:3], st_row[:, 3:4], "g")

            wm_bf = []
            for i in range(3):
                w32 = cp.tile([F, F], F32, tag=f"wm{i}_32")
                nc.sync.dma_start(out=w32[:],
                                  in_=wm.ap()[i * F : (i + 1) * F, :])
                wb = cp.tile([F, F], BF16, tag=f"wm{i}_bf")
                nc.vector.tensor_copy(out=wb[:], in_=w32[:])
                wm_bf.append(wb)
            bm_row = cp.tile([1, F], F32)
            nc.sync.dma_start(out=bm_row[:], in_=bm.ap())
            bm_ps = pp.tile([TILE, F], F32, space="PSUM", tag="pro")
            nc.tensor.matmul(out=bm_ps[:], lhsT=onesr[:], rhs=bm_row[:],
                             start=True, stop=True)
            bm_b = cp.tile([TILE, F], F32)
            nc.vector.tensor_copy(out=bm_b[:], in_=bm_ps[:])

            for t in range(NT):
                h2t = wp.tile([TILE, F], F32, tag="h2t")
                nc.sync.dma_start(out=h2t[:],
                                  in_=h2_sh.ap()[t * TILE : (t + 1) * TILE, :])
                t1 = wp.tile([TILE, F], F32, tag="t1")
                nc.vector.tensor_tensor(out=t1[:], in0=h2t[:], in1=s_b[:],
                                        op=ALU.mult)
                nc.vector.tensor_tensor(out=t1[:], in0=t1[:], in1=o_b[:],
                                        op=ALU.add)
                h2n = wp.tile([TILE, F], F32, tag="h2n")
                nc.scalar.activation(h2n[:], t1[:], ACTF.Relu)
                hT_ps = pt.tile([TILE, TILE], F32, space="PSUM", tag="tr")
                nc.tensor.transpose(out=hT_ps[:], in_=h2n[:], identity=idn[:])
                hT = wp.tile([TILE, TILE], BF16, tag="hTb")
                nc.vector.tensor_copy(out=hT[:], in_=hT_ps[:])

                dB_ps = pt.tile([TILE, TILE], F32, space="PSUM", tag="tr")
                nc.tensor.transpose(
                    out=dB_ps[:],
                    in_=dist_t[:, t : t + 1].to_broadcast([TILE, TILE]),
                    identity=idn[:])
                dfT = wp.tile([TILE, TILE], BF16, tag="dfT")
                nc.scalar.activation(dfT[:], dB_ps[:], ACTF.Relu,
                                     scale=ad_col[:], bias=bd_col[:])
                gB_ps = pt.tile([TILE, TILE], F32, space="PSUM", tag="tr")
                nc.tensor.transpose(
                    out=gB_ps[:],
                    in_=degf_t[:, t : t + 1].to_broadcast([TILE, TILE]),
                    identity=idn[:])
                gfT = wp.tile([TILE, TILE], BF16, tag="gfT")
                nc.scalar.activation(gfT[:], gB_ps[:], ACTF.Relu,
                                     scale=ag_col[:], bias=bg_col[:])

                o_ps = po.tile([TILE, F], F32, space="PSUM", tag="o")
                nc.tensor.matmul(out=o_ps[:], lhsT=hT[:], rhs=wm_bf[0][:],
                                 start=True, stop=False)
                nc.tensor.matmul(out=o_ps[:], lhsT=dfT[:], rhs=wm_bf[1][:],
                                 start=False, stop=False)
                nc.tensor.matmul(out=o_ps[:], lhsT=gfT[:], rhs=wm_bf[2][:],
                                 start=False, stop=True)
                ot = wp.tile([TILE, F], F32, tag="ot")
                nc.vector.tensor_tensor(out=ot[:], in0=o_ps[:], in1=bm_b[:],
                                        op=ALU.add)
                nc.sync.dma_start(out=out_sh.ap()[t * TILE : (t + 1) * TILE, :],
                                  in_=ot[:])
    nc.compile()
    return nc


# ----------------------------------------------------------------------------
# cached PJRT SPMD runner (no donation; device-resident inputs; wall timing)
# ----------------------------------------------------------------------------

_RUN_CACHE = {}
LAST_TIMINGS = {}


def _make_runner(nc):
    bass2jax.install_neuronx_cc_hook()
    partition_name = (nc.partition_id_tensor.name
                      if nc.partition_id_tensor else None)
    in_names, out_names, out_avals = [], [], []
    for alloc in nc.m.functions[0].allocations:
        if not isinstance(alloc, mybir.MemoryLocationSet):
            continue
        name = alloc.memorylocations[0].name
        if alloc.kind == "ExternalInput":
            if name != partition_name:
                in_names.append(name)
        elif alloc.kind == "ExternalOutput":
            out_names.append(name)
            out_avals.append(jax.core.ShapedArray(
                tuple(alloc.tensor_shape), mybir.dt.np(alloc.dtype)))
    n_params = len(in_names)
    all_names = in_names + out_names
    if partition_name is not None:
        all_names = all_names + [partition_name]

    def _body(*args):
        operands = list(args)
        if partition_name is not None:
            operands.append(bass2jax.partition_id_tensor())
        outs = bass2jax._bass_exec_p.bind(
            *operands,
            out_avals=tuple(out_avals),
            in_names=tuple(all_names),
            out_names=tuple(out_names),
            lowering_input_output_aliases=(),
            sim_require_finite=True,
            sim_require_nnan=True,
            nc=nc,
        )
        return tuple(outs)

    devices = jax.devices()[:NCORES]
    mesh = Mesh(np.asarray(devices), ("core",))
    sharded = jax.jit(shard_map(
        _body, mesh=mesh,
        in_specs=(PartitionSpec("core"),) * (n_params + len(out_names)),
        out_specs=(PartitionSpec("core"),) * len(out_names),
        check_rep=False))
    return sharded, in_names, out_names, out_avals, mesh


def _run(tag, nc, in_maps, time_it=False):
    key = id(nc)
    if key not in _RUN_CACHE:
        _RUN_CACHE[key] = _make_runner(nc)
    sharded, in_names, out_names, out_avals, mesh = _RUN_CACHE[key]

    concat_in = [
        np.concatenate([np.asarray(in_maps[c][n]) for c in range(NCORES)], axis=0)
        for n in in_names
    ]
    concat_zeros = [
        np.zeros((NCORES * a.shape[0],) + tuple(a.shape[1:]), a.dtype)
        for a in out_avals
    ]
    sh = jax.sharding.NamedSharding(mesh, PartitionSpec("core"))
    dev_in = [jax.device_put(a, sh) for a in concat_in]
    dev_zero = [jax.device_put(a, sh) for a in concat_zeros]
    out = sharded(*dev_in, *dev_zero)
    jax.block_until_ready(out)
    if time_it:
        # marginal per-call time from two pipelined batch sizes -- the first
        # call in a batch carries the RPC/dispatch sync, extra calls queue
        # back-to-back on the device.
        def batch(n):
            t0 = time.perf_counter()
            outs = [sharded(*dev_in, *dev_zero) for _ in range(n)]
            jax.block_until_ready(outs)
            return time.perf_counter() - t0
        batch(2)
        t_small = min(batch(3), batch(3))
        t_big = min(batch(19), batch(19))
        LAST_TIMINGS[tag] = max((t_big - t_small) / 16, 1e-6)
    res = [
        {n: np.asarray(out[i]).reshape((NCORES,) + out_avals[i].shape)[c]
         for i, n in enumerate(out_names)}
        for c in range(NCORES)
    ]
    return res


# ----------------------------------------------------------------------------
# kernel entry point
# ----------------------------------------------------------------------------

_PROG_CACHE = {}


def kernel(x, edge_index, edge_weight, dist_feat, degree_feat,
           W1, b1, W2, b2, bn1_g, bn1_b, bn2_g, bn2_b,
           Wd, bd, bnd_g, bnd_b, Wg, bg, bng_g, bng_b, Wm, bm,
           _time_launches=False):
    edge_index = np.asarray(edge_index)
    new_id = _relabel(edge_index)
    meta, arrays = _prep_edges(edge_index, np.asarray(edge_weight), new_id)

    mkey = (meta["TOTC"], meta["KDTOT"],
            tuple(meta["K_tg"].reshape(-1).tolist()))
    if mkey not in _PROG_CACHE:
        _PROG_CACHE.clear()
        _PROG_CACHE[mkey] = {
            "L0": _build_L0(meta),
            "conv": _build_conv(meta),
            "L2": _build_L2(meta),
            "L4": _build_L4(meta),
        }
    progs = _PROG_CACHE[mkey]

    x = np.asarray(x, np.float32)
    x_sh = _scatter_rows(x, new_id)
    dist_cols = _col_layout(np.asarray(dist_feat)[:, 0], new_id)
    degf_cols = _col_layout(np.asarray(degree_feat)[:, 0], new_id)
    ones_col = np.ones((TILE, 1), np.float32)
    ones_row = np.ones((1, TILE), np.float32)
    ident = np.eye(TILE, dtype=np.float32)
    iota = np.tile(np.arange(TILE, dtype=np.float32).astype(_bf)[None, :],
                   (TILE, 1))

    # ---- L0
    r0 = _run("L0", progs["L0"], [
        {"x_sh": x_sh[c], "ewn": arrays["ewn"][c], "dist_sh": dist_cols[c],
         "degf_sh": degf_cols[c], "ones_col": ones_col}
        for c in range(NCORES)
    ], time_it=_time_launches)
    dinv_own = np.stack([r0[c]["dinv_out"] for c in range(NCORES)])
    xp_full = np.concatenate(
        [r0[c]["xp_out"] for c in range(NCORES)])              # [NV, F] bf16
    st4 = np.stack([r0[c]["st4_out"][0] for c in range(NCORES)])  # [8, 4]

    # ---- L1 (conv1)
    conv_base = [
        {"idx_all": arrays["idx_all"][c], "ew_cols": arrays["ew_cols"][c],
         "dl_cols": arrays["dloc_cols"][c], "dinv": dinv_own[c],
         "iota_in": iota}
        for c in range(NCORES)
    ]
    W1f = np.asarray(W1, np.float32)
    r1 = _run("L1", progs["conv"], [
        dict(m, tbl=xp_full, w_in=W1f) for m in conv_base
    ], time_it=_time_launches)
    h1_sh = [r1[c]["h_out"] for c in range(NCORES)]
    sums1 = np.concatenate([r1[c]["sum_out"] for c in range(NCORES)])
    sqs1 = np.concatenate([r1[c]["sq_out"] for c in range(NCORES)])

    # ---- L2
    r2 = _run("L2", progs["L2"], [
        {"h1_sh": h1_sh[c], "sums": sums1, "sqs": sqs1,
         "bn_g": np.asarray(bn1_g, np.float32)[None, :],
         "bn_b": np.asarray(bn1_b, np.float32)[None, :],
         "dinv": dinv_own[c], "ones_col": ones_col, "ones_row": ones_row}
        for c in range(NCORES)
    ], time_it=_time_launches)
    h1nd_full = np.concatenate(
        [r2[c]["hn_out"] for c in range(NCORES)])              # [NV, F] bf16

    # ---- L3 (conv2, same program)
    W2f = np.asarray(W2, np.float32)
    r3 = _run("L3", progs["conv"], [
        dict(m, tbl=h1nd_full, w_in=W2f) for m in conv_base
    ], time_it=_time_launches)
    h2_sh = [r3[c]["h_out"] for c in range(NCORES)]
    sums2 = np.concatenate([r3[c]["sum_out"] for c in range(NCORES)])
    sqs2 = np.concatenate([r3[c]["sq_out"] for c in range(NCORES)])

    # ---- L4
    r4 = _run("L4", progs["L4"], [
        {"h2_sh": h2_sh[c], "sums": sums2, "sqs": sqs2,
         "bn_g": np.asarray(bn2_g, np.float32)[None, :],
         "bn_b": np.asarray(bn2_b, np.float32)[None, :],
         "st4": st4,
         "wd": np.asarray(Wd, np.float32).reshape(1, F),
         "bnd_g": np.asarray(bnd_g, np.float32)[None, :],
         "bnd_b": np.asarray(bnd_b, np.float32)[None, :],
         "wg": np.asarray(Wg, np.float32).reshape(1, F),
         "bng_g": np.asarray(bng_g, np.float32)[None, :],
         "bng_b": np.asarray(bng_b, np.float32)[None, :],
         "wm": np.asarray(Wm, np.float32), "bm": np.asarray(bm, np.float32)[None, :],
         "dist_sh": dist_cols[c], "degf_sh": degf_cols[c],
         "ones_col": ones_col, "ones_row": ones_row, "ident": ident}
        for c in range(NCORES)
    ], time_it=_time_launches)
    out_nv = np.concatenate([r4[c]["out_sh"] for c in range(NCORES)])
    return out_nv[new_id]
